# revision 1
# baseline (speedup 1.0000x reference)
"""Trainium2 Bass kernel for CrossAttentionModule (channel-wise attention).

Math restructuring
------------------
Reference (per sample b, with n = H*W pixels, C channels):
    q = Wq @ fm + bq            # [C, n]
    k = Wk * am + bk            # [C, n]  (rank-2 in the channel axis!)
    v = Wv @ fm + bv            # [C, n]
    scores[i, j] = <q[i, :], k[j, :]>
    out = softmax_j(scores) @ v
    result = gamma * out + fm

Because k[j, p] = Wk[j] * am[p] + bk[j]:
    scores[i, j] = s1[i] * Wk[j] + s2[i] * bk[j]
where
    s1 = Wq @ (fm @ am) + sum(am) * bq      # [C]
    s2 = Wq @ (fm @ 1)  + n * bq            # [C]
so the whole Q GEMM and the scores GEMM collapse into two C-vector
matvecs against Wq.  The softmax row max is max_j of a 2D linear
function over the point set {(Wk[j], bk[j])} -- we evaluate it over a
small set of direction-sampled support points (argmax over 64 angles,
precomputed on host from the weights; undershoot <= r*(1-cos(pi/64)),
harmless inside exp).  Z comes for free from the main matmul by
appending a ones-column to v.

Sharding: data-parallel over batch; core b computes sample b.
"""

import os
import sys

for _p in ("/opt/trn_rl_repo", "/root/.axon_site/_ro/trn_rl_repo"):
    if os.path.isdir(_p) and _p not in sys.path:
        sys.path.insert(0, _p)

from contextlib import ExitStack

import numpy as np

import concourse.bacc as bacc
import concourse.bass as bass
import concourse.mybir as mybir
import concourse.tile as tile

C = 2048
NPIX = 1024
NCORES = 8
NH = 64  # direction-sampled support points for the row max
NCHUNK = C // 128  # 16

F32 = mybir.dt.float32
OP = mybir.AluOpType
AX = mybir.AxisListType
AF = mybir.ActivationFunctionType

# dtype used for the two big GEMMs (V and probs@V).  float32r streams at
# full PE rate (4x faster than float32) at reduced multiply precision;
# the score/softmax path always stays full fp32.
MM_DT = mybir.dt.float32r if os.environ.get("CA_MM_DT", "f32r") == "f32r" else F32

# n-chunk split of the 1026-wide (v | ones | pad) moving operand: each
# matmul output must fit one PSUM bank (<=512 fp32), and f32r needs an
# even moving dim.  Column 1024 is the ones-column (Z); 1025 is padding.
NSPLIT = [(0, 342), (342, 684), (684, 1026)]


def build_nc(mm_dt=MM_DT, passes=1):
    nc = bacc.Bacc("TRN2", target_bir_lowering=False)

    fm = nc.declare_dram_parameter("fm", [C, NPIX], F32, isOutput=False)
    am = nc.declare_dram_parameter("am", [1, NPIX], F32, isOutput=False)
    # weight blocks pre-swizzled on host: [o, p, c, f] = W.T[c*128+p, o*128+f]
    wvt = nc.declare_dram_parameter("wvt", [NCHUNK, 128, NCHUNK, 128], F32, isOutput=False)
    wqt = nc.declare_dram_parameter("wqt", [NCHUNK, 128, NCHUNK, 128], F32, isOutput=False)
    # smalls[p, 16*k + o] = vec_k[o*128 + p] for vec_k in (wk, bk, bq, bv)
    smalls = nc.declare_dram_parameter("smalls", [128, 4 * NCHUNK], F32, isOutput=False)
    hull = nc.declare_dram_parameter("hull", [2, NH], F32, isOutput=False)
    gam = nc.declare_dram_parameter("gamma", [1, 1], F32, isOutput=False)
    out = nc.declare_dram_parameter("out", [C, NPIX], F32, isOutput=True)

    with ExitStack() as ctx:
        tc = ctx.enter_context(tile.TileContext(nc))
        small = ctx.enter_context(tc.tile_pool(name="small", bufs=1))
        vpool = ctx.enter_context(tc.tile_pool(name="v", bufs=NCHUNK))
        dramp = ctx.enter_context(tc.tile_pool(name="dram", bufs=1, space="DRAM"))

        # ---- small persistent tiles -------------------------------------
        am_bc = small.tile([128, NPIX], F32, tag="am_bc")
        nc.gpsimd.dma_start(out=am_bc[:], in_=am[:].to_broadcast([128, NPIX]))
        hull_wk = small.tile([128, NH], F32, tag="hwk")
        nc.gpsimd.dma_start(out=hull_wk[:], in_=hull[0:1, :].to_broadcast([128, NH]))
        hull_bk = small.tile([128, NH], F32, tag="hbk")
        nc.gpsimd.dma_start(out=hull_bk[:], in_=hull[1:2, :].to_broadcast([128, NH]))
        gam_bc = small.tile([128, 1], F32, tag="gam")
        nc.gpsimd.dma_start(out=gam_bc[:], in_=gam[:].to_broadcast([128, 1]))

        smalls_t = small.tile([128, 4 * NCHUNK], F32, tag="smalls_t")
        nc.sync.dma_start(out=smalls_t[:], in_=smalls[:])
        wk_t = smalls_t[:, 0:NCHUNK]
        bk_t = smalls_t[:, NCHUNK : 2 * NCHUNK]
        bq_t = smalls_t[:, 2 * NCHUNK : 3 * NCHUNK]
        bv_t = smalls_t[:, 3 * NCHUNK : 4 * NCHUNK]

        a_col = small.tile([128, 1], F32, tag="a_col")
        nc.vector.tensor_reduce(out=a_col[:], in_=am_bc[:], axis=AX.X, op=OP.add)

        # s1 in cols 0..15, s2 in 16..31, m in 32..47 (col o <-> i-chunk o)
        s_cols = small.tile([128, 3 * NCHUNK], F32, tag="s_cols")
        scratch = dramp.tile([3, C], F32, tag="scratch")

        # `passes` > 1 re-runs the whole pipeline for differential timing.
        for _pass in range(passes):
            v_tiles = []

            # ---- phase A/B: u reduction, V GEMM, s matvec, row-max ----------
            with ExitStack() as pab:
                fm_pool = pab.enter_context(tc.tile_pool(name="fm", bufs=NCHUNK))
                u_pool = pab.enter_context(tc.tile_pool(name="u", bufs=NCHUNK))
                wv_pool = pab.enter_context(tc.tile_pool(name="wv", bufs=2))
                wq_pool = pab.enter_context(tc.tile_pool(name="wq", bufs=2))
                scr_pool = pab.enter_context(tc.tile_pool(name="scr", bufs=1))
                hx_pool = pab.enter_context(tc.tile_pool(name="hx", bufs=2))
                psv = pab.enter_context(tc.tile_pool(name="psv", bufs=4, space="PSUM"))
                pss = pab.enter_context(tc.tile_pool(name="pss", bufs=2, space="PSUM"))

                fm_tiles = []
                u_tiles = []
                for c in range(NCHUNK):
                    # single exact-f32 read of fm; the f32r matmul copy is
                    # produced on-chip (ACT rounds on write), saving a second
                    # 8MB HBM stream.  The u reductions read the exact tile
                    # (score path must stay fp32 -- errors amplify via exp).
                    us = scr_pool.tile([128, NPIX], F32, tag="us", bufs=3)
                    nc.sync.dma_start(out=us[:], in_=fm[c * 128 : (c + 1) * 128, :])
                    ft = fm_pool.tile([128, NPIX], mm_dt, tag="fm")
                    nc.scalar.activation(out=ft[:], in_=us[:], func=AF.Copy)
                    fm_tiles.append(ft)
                    ut = u_pool.tile([128, 2], F32, tag="u")
                    scr_a = scr_pool.tile([128, NPIX], F32, tag="scr_a")
                    nc.vector.tensor_mul(scr_a[:], us[:], am_bc[:])
                    nc.vector.tensor_reduce(
                        out=ut[:, 0:1], in_=scr_a[:], axis=AX.X, op=OP.add
                    )
                    nc.vector.tensor_reduce(
                        out=ut[:, 1:2], in_=us[:], axis=AX.X, op=OP.add
                    )
                    u_tiles.append(ut)

                for o in range(NCHUNK):
                    # one 1MB DMA per o-chunk weight block
                    wvb = wv_pool.tile([128, NCHUNK, 128], mm_dt, tag="wv")
                    nc.sync.dma_start(out=wvb[:], in_=wvt[o].bitcast(mm_dt))
                    pv0 = psv.tile([128, 512], F32, tag="pv")
                    pv1 = psv.tile([128, 512], F32, tag="pv")
                    # c-inner with both p-halves per c: consecutive matmuls
                    # share the stationary operand
                    for c in range(NCHUNK):
                        nc.tensor.matmul(
                            pv0[:],
                            wvb[:, c, :],
                            fm_tiles[c][:, 0:512],
                            start=(c == 0),
                            stop=(c == NCHUNK - 1),
                        )
                        nc.tensor.matmul(
                            pv1[:],
                            wvb[:, c, :],
                            fm_tiles[c][:, 512:1024],
                            start=(c == 0),
                            stop=(c == NCHUNK - 1),
                        )
                    vt = vpool.tile([128, NPIX + 2], mm_dt, tag="v")
                    nc.scalar.activation(
                        out=vt[:, 0:512], in_=pv0[:], func=AF.Identity,
                        bias=bv_t[:, o : o + 1],
                    )
                    nc.scalar.activation(
                        out=vt[:, 512:1024], in_=pv1[:], func=AF.Identity,
                        bias=bv_t[:, o : o + 1],
                    )
                    nc.vector.memset(vt[:, 1024:1026].bitcast(F32), 1.0)
                    v_tiles.append(vt)

                    # s matvec for this o-chunk (always full fp32)
                    wqb = wq_pool.tile([128, NCHUNK, 128], F32, tag="wq")
                    nc.scalar.dma_start(out=wqb[:], in_=wqt[o])
                    ps = pss.tile([128, 2], F32, tag="ps")
                    for c in range(NCHUNK):
                        nc.tensor.matmul(
                            ps[:],
                            wqb[:, c, :],
                            u_tiles[c][:],
                            start=(c == 0),
                            stop=(c == NCHUNK - 1),
                        )
                    nc.vector.scalar_tensor_tensor(
                        out=s_cols[:, o : o + 1],
                        in0=bq_t[:, o : o + 1],
                        scalar=a_col[:, 0:1],
                        in1=ps[:, 0:1],
                        op0=OP.mult,
                        op1=OP.add,
                    )
                    nc.vector.scalar_tensor_tensor(
                        out=s_cols[:, NCHUNK + o : NCHUNK + o + 1],
                        in0=bq_t[:, o : o + 1],
                        scalar=float(NPIX),
                        in1=ps[:, 1:2],
                        op0=OP.mult,
                        op1=OP.add,
                    )
                    # row max via support points
                    hx = hx_pool.tile([128, NH], F32, tag="hx")
                    nc.vector.tensor_scalar_mul(hx[:], hull_wk[:], s_cols[:, o : o + 1])
                    nc.vector.scalar_tensor_tensor(
                        out=hx[:],
                        in0=hull_bk[:],
                        scalar=s_cols[:, NCHUNK + o : NCHUNK + o + 1],
                        in1=hx[:],
                        op0=OP.mult,
                        op1=OP.add,
                    )
                    nc.vector.tensor_reduce(
                        out=s_cols[:, 2 * NCHUNK + o : 2 * NCHUNK + o + 1],
                        in_=hx[:],
                        axis=AX.X,
                        op=OP.max,
                    )
                    # park the three columns in DRAM (partition-major = i order)
                    for r in range(3):
                        nc.sync.dma_start(
                            out=scratch[r : r + 1, o * 128 : (o + 1) * 128],
                            in_=s_cols[:, r * NCHUNK + o : r * NCHUNK + o + 1],
                        )

            # ---- phase D: exp(scores^T) blocks + probs @ v ------------------
            with ExitStack() as pd:
                rows = pd.enter_context(tc.tile_pool(name="rows", bufs=1))
                e_pool = pd.enter_context(tc.tile_pool(name="e", bufs=24))
                res_pool = pd.enter_context(tc.tile_pool(name="res", bufs=3))
                o_pool = pd.enter_context(tc.tile_pool(name="osb", bufs=3))
                z_pool = pd.enter_context(tc.tile_pool(name="z", bufs=4))
                pso = pd.enter_context(tc.tile_pool(name="pso", bufs=6, space="PSUM"))

                s1r = rows.tile([128, C], F32, tag="s1r")
                s2r = rows.tile([128, C], F32, tag="s2r")
                m_r = rows.tile([128, C], F32, tag="m_r")
                nc.sync.dma_start(out=s1r[:], in_=scratch[0:1, :].to_broadcast([128, C]))
                nc.sync.dma_start(out=s2r[:], in_=scratch[1:2, :].to_broadcast([128, C]))
                nc.sync.dma_start(out=m_r[:], in_=scratch[2:3, :].to_broadcast([128, C]))

                for ib in range(4):
                    isl = slice(ib * 512, (ib + 1) * 512)
                    eb = []
                    for j in range(NCHUNK):
                        es = e_pool.tile([128, 512], F32, tag="escr", bufs=3)
                        # (s2_i * bk_j) - m_i  (walrus only codegens this
                        # op on DVE; Pool-engine offload fails NCC_IXCG966)
                        nc.vector.scalar_tensor_tensor(
                            out=es[:],
                            in0=s2r[:, isl],
                            scalar=bk_t[:, j : j + 1],
                            in1=m_r[:, isl],
                            op0=OP.mult,
                            op1=OP.subtract,
                        )
                        # (s1_i * wk_j) + prev
                        nc.vector.scalar_tensor_tensor(
                            out=es[:],
                            in0=s1r[:, isl],
                            scalar=wk_t[:, j : j + 1],
                            in1=es[:],
                            op0=OP.mult,
                            op1=OP.add,
                        )
                        # exp writes the (rounded) matmul operand dtype
                        et = e_pool.tile([128, 512], mm_dt, tag="e")
                        nc.scalar.activation(out=et[:], in_=es[:], func=AF.Exp)
                        eb.append(et)
                    for ic in range(4):
                        ig = ib * 4 + ic
                        po = [
                            pso.tile([128, b - a], F32, tag="po", name=f"po{_pass}_{ig}_{a}")
                            for (a, b) in NSPLIT
                        ]
                        # j-outer so the three n-chunk matmuls reuse the
                        # same stationary operand (one weight load per j)
                        for j in range(NCHUNK):
                            for nidx, (a, b) in enumerate(NSPLIT):
                                nc.tensor.matmul(
                                    po[nidx][:],
                                    eb[j][:, ic * 128 : (ic + 1) * 128],
                                    v_tiles[j][:, a:b],
                                    start=(j == 0),
                                    stop=(j == NCHUNK - 1),
                                )
                        rz = z_pool.tile([128, 1], F32, tag="rz")
                        nc.vector.reciprocal(rz[:], po[2][:, 340:341])
                        rzg = z_pool.tile([128, 1], F32, tag="rzg")
                        nc.vector.tensor_mul(rzg[:], rz[:], gam_bc[:])
                        fr = res_pool.tile([128, NPIX], F32, tag="res")
                        nc.scalar.dma_start(
                            out=fr[:], in_=fm[ig * 128 : (ig + 1) * 128, :]
                        )
                        ot = o_pool.tile([128, NPIX], F32, tag="osb")
                        spans = [(0, 342, 0), (342, 684, 1), (684, 1024, 2)]
                        for a, b, nidx in spans:
                            nc.vector.scalar_tensor_tensor(
                                out=ot[:, a:b],
                                in0=po[nidx][:, 0 : b - a],
                                scalar=rzg[:, 0:1],
                                in1=fr[:, a:b],
                                op0=OP.mult,
                                op1=OP.add,
                            )
                        nc.sync.dma_start(
                            out=out[ig * 128 : (ig + 1) * 128, :], in_=ot[:]
                        )

    nc.compile()
    return nc


def host_inputs(feature_map, attention_map, Wq, bq, Wk, bk, Wv, bv, gamma):
    """Shard + lay out inputs for the 8 cores; returns in_maps list."""
    f32 = np.float32
    B = feature_map.shape[0]
    fm = np.ascontiguousarray(feature_map.reshape(B, C, NPIX).astype(f32, copy=False))
    am = np.ascontiguousarray(
        attention_map.reshape(B, 1, NPIX).astype(f32, copy=False)
    )
    # blk[o, p, c, f] = W.T[c*128+p, o*128+f] = W[o*128+f, c*128+p]
    wqt_blk = np.ascontiguousarray(
        Wq.astype(f32, copy=False)
        .reshape(NCHUNK, 128, NCHUNK, 128)
        .transpose(0, 3, 2, 1)
    )
    wvt_blk = np.ascontiguousarray(
        Wv.astype(f32, copy=False)
        .reshape(NCHUNK, 128, NCHUNK, 128)
        .transpose(0, 3, 2, 1)
    )
    wk1 = Wk.reshape(C).astype(f32, copy=False)
    bk1 = bk.reshape(C).astype(f32, copy=False)
    # smalls[p, 16*k + o] = vec_k[o*128 + p]
    smalls = np.ascontiguousarray(
        np.concatenate(
            [
                v.reshape(C).astype(f32, copy=False).reshape(NCHUNK, 128).T
                for v in (Wk, bk, bq, bv)
            ],
            axis=1,
        )
    )

    # direction-sampled support points of {(Wk_j, bk_j)}: subset whose max
    # of (Wk_j * x + bk_j * y) is within r*(1-cos(pi/NH)) of the true max
    th = np.arange(NH, dtype=np.float64) * (2.0 * np.pi / NH)
    proj = np.cos(th)[:, None] * wk1[None, :] + np.sin(th)[:, None] * bk1[None, :]
    sel = np.argmax(proj, axis=1)
    hull = np.ascontiguousarray(np.stack([wk1[sel], bk1[sel]]).astype(f32))

    gam2 = np.ascontiguousarray(gamma.reshape(1, 1).astype(f32, copy=False))

    shared = dict(
        wvt=wvt_blk,
        wqt=wqt_blk,
        smalls=smalls,
        hull=hull,
        gamma=gam2,
    )
    return [dict(fm=fm[b], am=am[b], **shared) for b in range(B)]


_NC_CACHE = {}


def get_nc(mm_dt=MM_DT):
    key = str(mm_dt)
    if key not in _NC_CACHE:
        _NC_CACHE[key] = build_nc(mm_dt)
    return _NC_CACHE[key]


def kernel(feature_map, attention_map, Wq, bq, Wk, bk, Wv, bv, gamma, **run_kwargs):
    from concourse.bass_utils import run_bass_kernel_spmd

    # plain numpy up front (jax-array inputs would run host prep on device)
    feature_map, attention_map, Wq, bq, Wk, bk, Wv, bv, gamma = (
        np.asarray(x) for x in (feature_map, attention_map, Wq, bq, Wk, bk, Wv, bv, gamma)
    )
    B, _, H, W = feature_map.shape
    in_maps = host_inputs(
        feature_map, attention_map, Wq, bq, Wk, bk, Wv, bv, gamma
    )
    nc = get_nc()
    res = run_bass_kernel_spmd(nc, in_maps, core_ids=list(range(NCORES)), **run_kwargs)
    out = np.stack([res.results[b]["out"].reshape(C, H, W) for b in range(B)])
    if run_kwargs:
        kernel.last_results = res
    return out.astype(np.float32, copy=False)



# revision 5
# speedup vs baseline: 3.4630x; 3.4630x over previous
"""Trainium2 Bass kernel for CrossAttentionModule (channel-wise attention).

Math restructuring
------------------
Reference (per sample b, with n = H*W pixels, C channels):
    q = Wq @ fm + bq            # [C, n]
    k = Wk * am + bk            # [C, n]  (rank-2 in the channel axis!)
    v = Wv @ fm + bv            # [C, n]
    scores[i, j] = <q[i, :], k[j, :]>
    out = softmax_j(scores) @ v
    result = gamma * out + fm

Because k[j, p] = Wk[j] * am[p] + bk[j]:
    scores[i, j] = s1[i] * Wk[j] + s2[i] * bk[j]
where
    s1 = Wq @ (fm @ am) + sum(am) * bq      # [C]
    s2 = Wq @ (fm @ 1)  + n * bq            # [C]
so the whole Q GEMM and the scores GEMM collapse into two C-vector
matvecs against Wq.  The softmax row max is max_j of a 2D linear
function over the point set {(Wk[j], bk[j])} -- we evaluate it over a
small set of direction-sampled support points (argmax over 64 angles,
precomputed on host from the weights; undershoot <= r*(1-cos(pi/64)),
harmless inside exp).  Z comes for free from the main matmul by
appending a ones-column to v.

Sharding: data-parallel over batch; core b computes sample b.
"""

import os
import sys

for _p in ("/opt/trn_rl_repo", "/root/.axon_site/_ro/trn_rl_repo"):
    if os.path.isdir(_p) and _p not in sys.path:
        sys.path.insert(0, _p)

from contextlib import ExitStack

import numpy as np

import concourse.bacc as bacc
import concourse.bass as bass
import concourse.mybir as mybir
import concourse.tile as tile

C = 2048
NPIX = 1024
NCORES = 8
NH = 64  # direction-sampled support points for the row max
NCHUNK = C // 128  # 16

F32 = mybir.dt.float32
OP = mybir.AluOpType
AX = mybir.AxisListType
AF = mybir.ActivationFunctionType

# dtype used for the two big GEMMs (V and probs@V).  float32r streams at
# full PE rate (4x faster than float32) at reduced multiply precision;
# the score/softmax path always stays full fp32.
MM_DT = mybir.dt.float32r if os.environ.get("CA_MM_DT", "f32r") == "f32r" else F32

# n-chunk split of the 1026-wide (v | ones | pad) moving operand: each
# matmul output must fit one PSUM bank (<=512 fp32), and f32r needs an
# even moving dim.  Column 1024 is the ones-column (Z); 1025 is padding.
NSPLIT = [(0, 342), (342, 684), (684, 1026)]


def build_nc(mm_dt=MM_DT, passes=1):
    nc = bacc.Bacc("TRN2", target_bir_lowering=False)

    fm = nc.declare_dram_parameter("fm", [C, NPIX], F32, isOutput=False)
    am = nc.declare_dram_parameter("am", [1, NPIX], F32, isOutput=False)
    # weight blocks pre-swizzled on host: [o, p, c, f] = W.T[c*128+p, o*128+f]
    wvt = nc.declare_dram_parameter("wvt", [NCHUNK, 128, NCHUNK, 128], F32, isOutput=False)
    wqt = nc.declare_dram_parameter("wqt", [NCHUNK, 128, NCHUNK, 128], F32, isOutput=False)
    # smalls[p, 16*k + o] = vec_k[o*128 + p] for vec_k in (wk, bk, bq, bv)
    smalls = nc.declare_dram_parameter("smalls", [128, 4 * NCHUNK], F32, isOutput=False)
    hull = nc.declare_dram_parameter("hull", [2, NH], F32, isOutput=False)
    gam = nc.declare_dram_parameter("gamma", [1, 1], F32, isOutput=False)
    out = nc.declare_dram_parameter("out", [C, NPIX], F32, isOutput=True)

    with ExitStack() as ctx:
        tc = ctx.enter_context(tile.TileContext(nc))
        small = ctx.enter_context(tc.tile_pool(name="small", bufs=1))
        vpool = ctx.enter_context(tc.tile_pool(name="v", bufs=NCHUNK))
        dramp = ctx.enter_context(tc.tile_pool(name="dram", bufs=1, space="DRAM"))

        # ---- small persistent tiles -------------------------------------
        am_bc = small.tile([128, NPIX], F32, tag="am_bc")
        nc.gpsimd.dma_start(out=am_bc[:], in_=am[:].to_broadcast([128, NPIX]))
        hull_wk = small.tile([128, NH], F32, tag="hwk")
        nc.gpsimd.dma_start(out=hull_wk[:], in_=hull[0:1, :].to_broadcast([128, NH]))
        hull_bk = small.tile([128, NH], F32, tag="hbk")
        nc.gpsimd.dma_start(out=hull_bk[:], in_=hull[1:2, :].to_broadcast([128, NH]))
        gam_bc = small.tile([128, 1], F32, tag="gam")
        nc.gpsimd.dma_start(out=gam_bc[:], in_=gam[:].to_broadcast([128, 1]))

        smalls_t = small.tile([128, 4 * NCHUNK], F32, tag="smalls_t")
        nc.sync.dma_start(out=smalls_t[:], in_=smalls[:])
        wk_t = smalls_t[:, 0:NCHUNK]
        bk_t = smalls_t[:, NCHUNK : 2 * NCHUNK]
        bq_t = smalls_t[:, 2 * NCHUNK : 3 * NCHUNK]
        bv_t = smalls_t[:, 3 * NCHUNK : 4 * NCHUNK]

        a_col = small.tile([128, 1], F32, tag="a_col")
        nc.vector.tensor_reduce(out=a_col[:], in_=am_bc[:], axis=AX.X, op=OP.add)

        # s1 in cols 0..15, s2 in 16..31, m in 32..47 (col o <-> i-chunk o)
        s_cols = small.tile([128, 3 * NCHUNK], F32, tag="s_cols")
        scratch = dramp.tile([3, C], F32, tag="scratch")

        # `passes` > 1 re-runs the whole pipeline for differential timing.
        for _pass in range(passes):
            v_tiles = []

            # ---- phase A/B: u reduction, V GEMM, s matvec, row-max ----------
            with ExitStack() as pab:
                fm_pool = pab.enter_context(tc.tile_pool(name="fm", bufs=NCHUNK))
                u_pool = pab.enter_context(tc.tile_pool(name="u", bufs=NCHUNK))
                wv_pool = pab.enter_context(tc.tile_pool(name="wv", bufs=2))
                wq_pool = pab.enter_context(tc.tile_pool(name="wq", bufs=2))
                scr_pool = pab.enter_context(tc.tile_pool(name="scr", bufs=1))
                hx_pool = pab.enter_context(tc.tile_pool(name="hx", bufs=2))
                psv = pab.enter_context(tc.tile_pool(name="psv", bufs=4, space="PSUM"))
                pss = pab.enter_context(tc.tile_pool(name="pss", bufs=2, space="PSUM"))

                fm_tiles = []
                u_tiles = []
                for c in range(NCHUNK):
                    # single exact-f32 read of fm; the f32r matmul copy is
                    # produced on-chip (ACT rounds on write), saving a second
                    # 8MB HBM stream.  The u reductions read the exact tile
                    # (score path must stay fp32 -- errors amplify via exp).
                    us = scr_pool.tile([128, NPIX], F32, tag="us", bufs=3)
                    nc.sync.dma_start(out=us[:], in_=fm[c * 128 : (c + 1) * 128, :])
                    ft = fm_pool.tile([128, NPIX], mm_dt, tag="fm")
                    nc.scalar.activation(out=ft[:], in_=us[:], func=AF.Copy)
                    fm_tiles.append(ft)
                    ut = u_pool.tile([128, 2], F32, tag="u")
                    scr_a = scr_pool.tile([128, NPIX], F32, tag="scr_a")
                    nc.vector.tensor_mul(scr_a[:], us[:], am_bc[:])
                    nc.vector.tensor_reduce(
                        out=ut[:, 0:1], in_=scr_a[:], axis=AX.X, op=OP.add
                    )
                    nc.vector.tensor_reduce(
                        out=ut[:, 1:2], in_=us[:], axis=AX.X, op=OP.add
                    )
                    u_tiles.append(ut)

                for o in range(NCHUNK):
                    # one 1MB DMA per o-chunk weight block
                    wvb = wv_pool.tile([128, NCHUNK, 128], mm_dt, tag="wv")
                    nc.sync.dma_start(out=wvb[:], in_=wvt[o].bitcast(mm_dt))
                    pv0 = psv.tile([128, 512], F32, tag="pv")
                    pv1 = psv.tile([128, 512], F32, tag="pv")
                    # c-inner with both p-halves per c: consecutive matmuls
                    # share the stationary operand
                    for c in range(NCHUNK):
                        nc.tensor.matmul(
                            pv0[:],
                            wvb[:, c, :],
                            fm_tiles[c][:, 0:512],
                            start=(c == 0),
                            stop=(c == NCHUNK - 1),
                        )
                        nc.tensor.matmul(
                            pv1[:],
                            wvb[:, c, :],
                            fm_tiles[c][:, 512:1024],
                            start=(c == 0),
                            stop=(c == NCHUNK - 1),
                        )
                    vt = vpool.tile([128, NPIX + 2], mm_dt, tag="v")
                    nc.scalar.activation(
                        out=vt[:, 0:512], in_=pv0[:], func=AF.Identity,
                        bias=bv_t[:, o : o + 1],
                    )
                    nc.scalar.activation(
                        out=vt[:, 512:1024], in_=pv1[:], func=AF.Identity,
                        bias=bv_t[:, o : o + 1],
                    )
                    nc.vector.memset(vt[:, 1024:1026].bitcast(F32), 1.0)
                    v_tiles.append(vt)

                    # s matvec for this o-chunk (always full fp32)
                    wqb = wq_pool.tile([128, NCHUNK, 128], F32, tag="wq")
                    nc.scalar.dma_start(out=wqb[:], in_=wqt[o])
                    ps = pss.tile([128, 2], F32, tag="ps")
                    for c in range(NCHUNK):
                        nc.tensor.matmul(
                            ps[:],
                            wqb[:, c, :],
                            u_tiles[c][:],
                            start=(c == 0),
                            stop=(c == NCHUNK - 1),
                        )
                    nc.vector.scalar_tensor_tensor(
                        out=s_cols[:, o : o + 1],
                        in0=bq_t[:, o : o + 1],
                        scalar=a_col[:, 0:1],
                        in1=ps[:, 0:1],
                        op0=OP.mult,
                        op1=OP.add,
                    )
                    nc.vector.scalar_tensor_tensor(
                        out=s_cols[:, NCHUNK + o : NCHUNK + o + 1],
                        in0=bq_t[:, o : o + 1],
                        scalar=float(NPIX),
                        in1=ps[:, 1:2],
                        op0=OP.mult,
                        op1=OP.add,
                    )
                    # row max via support points
                    hx = hx_pool.tile([128, NH], F32, tag="hx")
                    nc.vector.tensor_scalar_mul(hx[:], hull_wk[:], s_cols[:, o : o + 1])
                    nc.vector.scalar_tensor_tensor(
                        out=hx[:],
                        in0=hull_bk[:],
                        scalar=s_cols[:, NCHUNK + o : NCHUNK + o + 1],
                        in1=hx[:],
                        op0=OP.mult,
                        op1=OP.add,
                    )
                    nc.vector.tensor_reduce(
                        out=s_cols[:, 2 * NCHUNK + o : 2 * NCHUNK + o + 1],
                        in_=hx[:],
                        axis=AX.X,
                        op=OP.max,
                    )
                    # park the three columns in DRAM (partition-major = i order)
                    for r in range(3):
                        nc.sync.dma_start(
                            out=scratch[r : r + 1, o * 128 : (o + 1) * 128],
                            in_=s_cols[:, r * NCHUNK + o : r * NCHUNK + o + 1],
                        )

            # ---- phase D: exp(scores^T) blocks + probs @ v ------------------
            with ExitStack() as pd:
                rows = pd.enter_context(tc.tile_pool(name="rows", bufs=1))
                e_pool = pd.enter_context(tc.tile_pool(name="e", bufs=24))
                res_pool = pd.enter_context(tc.tile_pool(name="res", bufs=3))
                o_pool = pd.enter_context(tc.tile_pool(name="osb", bufs=3))
                z_pool = pd.enter_context(tc.tile_pool(name="z", bufs=4))
                pso = pd.enter_context(tc.tile_pool(name="pso", bufs=6, space="PSUM"))

                s1r = rows.tile([128, C], F32, tag="s1r")
                s2r = rows.tile([128, C], F32, tag="s2r")
                m_r = rows.tile([128, C], F32, tag="m_r")
                nc.sync.dma_start(out=s1r[:], in_=scratch[0:1, :].to_broadcast([128, C]))
                nc.sync.dma_start(out=s2r[:], in_=scratch[1:2, :].to_broadcast([128, C]))
                nc.sync.dma_start(out=m_r[:], in_=scratch[2:3, :].to_broadcast([128, C]))

                for ib in range(4):
                    isl = slice(ib * 512, (ib + 1) * 512)
                    eb = []
                    for j in range(NCHUNK):
                        es = e_pool.tile([128, 512], F32, tag="escr", bufs=3)
                        # (s2_i * bk_j) - m_i  (walrus only codegens this
                        # op on DVE; Pool-engine offload fails NCC_IXCG966)
                        nc.vector.scalar_tensor_tensor(
                            out=es[:],
                            in0=s2r[:, isl],
                            scalar=bk_t[:, j : j + 1],
                            in1=m_r[:, isl],
                            op0=OP.mult,
                            op1=OP.subtract,
                        )
                        # (s1_i * wk_j) + prev
                        nc.vector.scalar_tensor_tensor(
                            out=es[:],
                            in0=s1r[:, isl],
                            scalar=wk_t[:, j : j + 1],
                            in1=es[:],
                            op0=OP.mult,
                            op1=OP.add,
                        )
                        # exp writes the (rounded) matmul operand dtype
                        et = e_pool.tile([128, 512], mm_dt, tag="e")
                        nc.scalar.activation(out=et[:], in_=es[:], func=AF.Exp)
                        eb.append(et)
                    for ic in range(4):
                        ig = ib * 4 + ic
                        po = [
                            pso.tile([128, b - a], F32, tag="po", name=f"po{_pass}_{ig}_{a}")
                            for (a, b) in NSPLIT
                        ]
                        # j-outer so the three n-chunk matmuls reuse the
                        # same stationary operand (one weight load per j)
                        for j in range(NCHUNK):
                            for nidx, (a, b) in enumerate(NSPLIT):
                                nc.tensor.matmul(
                                    po[nidx][:],
                                    eb[j][:, ic * 128 : (ic + 1) * 128],
                                    v_tiles[j][:, a:b],
                                    start=(j == 0),
                                    stop=(j == NCHUNK - 1),
                                )
                        rz = z_pool.tile([128, 1], F32, tag="rz")
                        nc.vector.reciprocal(rz[:], po[2][:, 340:341])
                        rzg = z_pool.tile([128, 1], F32, tag="rzg")
                        nc.vector.tensor_mul(rzg[:], rz[:], gam_bc[:])
                        fr = res_pool.tile([128, NPIX], F32, tag="res")
                        nc.scalar.dma_start(
                            out=fr[:], in_=fm[ig * 128 : (ig + 1) * 128, :]
                        )
                        ot = o_pool.tile([128, NPIX], F32, tag="osb")
                        spans = [(0, 342, 0), (342, 684, 1), (684, 1024, 2)]
                        for a, b, nidx in spans:
                            nc.vector.scalar_tensor_tensor(
                                out=ot[:, a:b],
                                in0=po[nidx][:, 0 : b - a],
                                scalar=rzg[:, 0:1],
                                in1=fr[:, a:b],
                                op0=OP.mult,
                                op1=OP.add,
                            )
                        nc.sync.dma_start(
                            out=out[ig * 128 : (ig + 1) * 128, :], in_=ot[:]
                        )

    nc.compile()
    return nc


def host_inputs(feature_map, attention_map, Wq, bq, Wk, bk, Wv, bv, gamma):
    """Shard + lay out inputs for the 8 cores; returns in_maps list."""
    f32 = np.float32
    B = feature_map.shape[0]
    fm = np.ascontiguousarray(feature_map.reshape(B, C, NPIX).astype(f32, copy=False))
    am = np.ascontiguousarray(
        attention_map.reshape(B, 1, NPIX).astype(f32, copy=False)
    )
    # blk[o, p, c, f] = W.T[c*128+p, o*128+f] = W[o*128+f, c*128+p]
    wqt_blk = np.ascontiguousarray(
        Wq.astype(f32, copy=False)
        .reshape(NCHUNK, 128, NCHUNK, 128)
        .transpose(0, 3, 2, 1)
    )
    wvt_blk = np.ascontiguousarray(
        Wv.astype(f32, copy=False)
        .reshape(NCHUNK, 128, NCHUNK, 128)
        .transpose(0, 3, 2, 1)
    )
    wk1 = Wk.reshape(C).astype(f32, copy=False)
    bk1 = bk.reshape(C).astype(f32, copy=False)
    # smalls[p, 16*k + o] = vec_k[o*128 + p]
    smalls = np.ascontiguousarray(
        np.concatenate(
            [
                v.reshape(C).astype(f32, copy=False).reshape(NCHUNK, 128).T
                for v in (Wk, bk, bq, bv)
            ],
            axis=1,
        )
    )

    # direction-sampled support points of {(Wk_j, bk_j)}: subset whose max
    # of (Wk_j * x + bk_j * y) is within r*(1-cos(pi/NH)) of the true max
    th = np.arange(NH, dtype=np.float64) * (2.0 * np.pi / NH)
    proj = np.cos(th)[:, None] * wk1[None, :] + np.sin(th)[:, None] * bk1[None, :]
    sel = np.argmax(proj, axis=1)
    hull = np.ascontiguousarray(np.stack([wk1[sel], bk1[sel]]).astype(f32))

    gam2 = np.ascontiguousarray(gamma.reshape(1, 1).astype(f32, copy=False))

    shared = dict(
        wvt=wvt_blk,
        wqt=wqt_blk,
        smalls=smalls,
        hull=hull,
        gamma=gam2,
    )
    return [dict(fm=fm[b], am=am[b], **shared) for b in range(B)]


_NC_CACHE = {}


def get_nc(mm_dt=MM_DT):
    key = str(mm_dt)
    if key not in _NC_CACHE:
        _NC_CACHE[key] = build_nc(mm_dt)
    return _NC_CACHE[key]


def kernel(feature_map, attention_map, Wq, bq, Wk, bk, Wv, bv, gamma, **run_kwargs):
    from concourse.bass_utils import run_bass_kernel_spmd

    # plain numpy up front (jax-array inputs would run host prep on device)
    feature_map, attention_map, Wq, bq, Wk, bk, Wv, bv, gamma = (
        np.asarray(x) for x in (feature_map, attention_map, Wq, bq, Wk, bk, Wv, bv, gamma)
    )
    B, _, H, W = feature_map.shape
    in_maps = host_inputs(
        feature_map, attention_map, Wq, bq, Wk, bk, Wv, bv, gamma
    )
    nc = get_nc()
    res = run_bass_kernel_spmd(nc, in_maps, core_ids=list(range(NCORES)), **run_kwargs)
    out = np.stack([res.results[b]["out"].reshape(C, H, W) for b in range(B)])
    if run_kwargs:
        kernel.last_results = res
    return out.astype(np.float32, copy=False)



# revision 6
# speedup vs baseline: 3.7432x; 1.0809x over previous
"""Trainium2 Bass kernel for CrossAttentionModule (channel-wise attention).

Math restructuring (as the v1 baseline): k is rank-2 in the channel
axis (k[j,p] = Wk[j]*am[p] + bk[j]), so scores[i,j] collapses to
s1[i]*Wk[j] + s2[i]*bk[j] with s = Wq @ (fm@[am,1]) + coef*bq -- the
whole Q GEMM becomes a 2-column matvec, and softmax row-maxes come from
direction-sampled support points of {(Wk[j], bk[j])}.

Engineering, driven by measured HW bottlenecks (DMA queues ~100 GB/s
each; fp32 matmul 4-6x slower than 16/8-bit; DVE the critical engine):

  * V GEMM and probs@V in fp8e4m3 DoubleRow (two channel planes per PE
    pass); Wv ships as fp8 (x64 prescale), fm/Wq/out as fp16.  The
    score/softmax arithmetic stays fp32.
  * s matvec computed as u^T @ Wq^T with the tiny u as the stationary
    operand: 16 two-column weight loads instead of 256 full 128x128
    loads, and s comes out in row layout [2, C] directly.
  * Row-max via hull support points in partitions + gpsimd
    partition_all_reduce; s rows replicated with partition_broadcast
    (no DRAM scratch roundtrip).
  * exp pipeline on full [128, C] rows; fm stays resident in SBUF so
    the residual add needs no second fm read; DMA spread over the
    SP/ACT/Pool queues.

Sharding: data-parallel over batch; core b computes sample b.
"""

import os
import sys

for _p in ("/opt/trn_rl_repo", "/root/.axon_site/_ro/trn_rl_repo"):
    if os.path.isdir(_p) and _p not in sys.path:
        sys.path.insert(0, _p)

from contextlib import ExitStack

import numpy as np

import concourse.bacc as bacc
import concourse.bass as bass
import concourse.bass_isa as bass_isa
import concourse.mybir as mybir
import concourse.tile as tile

C = 2048
NPIX = 1024
NCORES = 8
NH = 64  # direction-sampled support points for the row max
NCHUNK = C // 128  # 16
NCP = C // 256  # 8 channel pair-chunks for DoubleRow

F32 = mybir.dt.float32
FP16 = mybir.dt.float16
FP8 = mybir.dt.float8e4
OP = mybir.AluOpType
AX = mybir.AxisListType
AF = mybir.ActivationFunctionType
PM = mybir.MatmulPerfMode

WV_SCALE = 64.0  # Wv prescale so fp8e4m3 sees ~N(0,1.4) instead of N(0,0.02)

# v8 tiles hold (v | ones@1024 | pad) as [128, 2, VF8]; VF8 is padded to a
# multiple of 4 bytes per plane so the ones-memset can go through a f32
# bitcast.  probs@V output columns: 0..1023 pixels, 1024 = Z.
VF8 = 1032
NSPLIT8 = [(0, 412), (412, 824), (824, VF8)]  # out-col chunks, each <= 512

_ONES_PAT = float(np.frombuffer(bytes([0x38]) * 4, np.float32)[0])  # fp8 1.0 x4

POOL_U = os.environ.get("CA_POOL_U", "1") == "1"


def build_nc(passes=1):
    nc = bacc.Bacc("TRN2", target_bir_lowering=False)

    fm16 = nc.declare_dram_parameter("fm16", [C, NPIX], FP16, isOutput=False)
    am16 = nc.declare_dram_parameter("am16", [1, NPIX], FP16, isOutput=False)
    # wv8[o, p, cp, i, f] = fp8(WV_SCALE * Wv[o*128+f, cp*256+i*128+p])
    wv8 = nc.declare_dram_parameter("wv8", [NCHUNK, 128, NCP, 2, 128], FP8, isOutput=False)
    # wqT16[c, p, x] = fp16(Wq[x, c*128+p])  (Wq transposed, row-chunked)
    wqT16 = nc.declare_dram_parameter("wqT16", [NCHUNK, 128, C], FP16, isOutput=False)
    # bq duplicated on two partition rows, [2, C]
    bqrow = nc.declare_dram_parameter("bqrow", [2, C], F32, isOutput=False)
    # smalls[p, 16*k + o] = vec_k[o*128 + p] for vec_k in (wk, bk, bq, bv)
    smalls = nc.declare_dram_parameter("smalls", [128, 4 * NCHUNK], F32, isOutput=False)
    # hullc[p, :] = (wk, bk) of hull support point p (p>=NH repeats point 0)
    hullc = nc.declare_dram_parameter("hullc", [128, 2], F32, isOutput=False)
    gam = nc.declare_dram_parameter("gamma", [1, 1], F32, isOutput=False)
    out16 = nc.declare_dram_parameter("out16", [C, NPIX], FP16, isOutput=True)

    ueng = nc.gpsimd if POOL_U else nc.vector

    with ExitStack() as ctx:
        tc = ctx.enter_context(tile.TileContext(nc))
        small = ctx.enter_context(tc.tile_pool(name="small", bufs=1))
        uspool = ctx.enter_context(tc.tile_pool(name="us", bufs=NCHUNK))
        fm8pool = ctx.enter_context(tc.tile_pool(name="fm8", bufs=NCP))
        vpool = ctx.enter_context(tc.tile_pool(name="v8", bufs=NCHUNK // 2))
        srow_pool = ctx.enter_context(tc.tile_pool(name="srow", bufs=2))

        # ---- small persistent tiles -------------------------------------
        am_bc = small.tile([128, NPIX], FP16, tag="am_bc")
        nc.gpsimd.dma_start(out=am_bc[:], in_=am16[:].to_broadcast([128, NPIX]))
        hullc_t = small.tile([128, 2], F32, tag="hullc")
        nc.sync.dma_start(out=hullc_t[:], in_=hullc[:])
        gam_bc = small.tile([128, 1], F32, tag="gam")
        nc.gpsimd.dma_start(out=gam_bc[:], in_=gam[:].to_broadcast([128, 1]))

        smalls_t = small.tile([128, 4 * NCHUNK], F32, tag="smalls_t")
        nc.sync.dma_start(out=smalls_t[:], in_=smalls[:])
        wk_t = smalls_t[:, 0:NCHUNK]
        bk_t = smalls_t[:, NCHUNK : 2 * NCHUNK]
        bv_t = smalls_t[:, 3 * NCHUNK : 4 * NCHUNK]

        bqr_t = small.tile([2, C], F32, tag="bqr")
        nc.sync.dma_start(out=bqr_t[:], in_=bqrow[:])

        a_col = small.tile([128, 1], F32, tag="a_col")
        nc.vector.tensor_reduce(out=a_col[:], in_=am_bc[:], axis=AX.X, op=OP.add)
        # scalar2 = (sum(am), NPIX) as a [2,1] per-partition scalar
        scalar2 = small.tile([2, 1], F32, tag="sc2")
        nc.vector.memset(scalar2[:], float(NPIX))
        nc.scalar.activation(out=scalar2[0:1, :], in_=a_col[0:1, :], func=AF.Copy)

        for _pass in range(passes):
            us_tiles = []
            fm8_tiles = []
            v8_tiles = []

            srow = srow_pool.tile([2, C], F32, tag="srow")

            # ---- phase A: fm load, fp8 copy, u reductions -------------------
            with ExitStack() as pa:
                u_pool = pa.enter_context(tc.tile_pool(name="u", bufs=NCHUNK))
                scr_pool = pa.enter_context(tc.tile_pool(name="scr", bufs=2))

                u16_tiles = []
                for cp in range(NCP):
                    f8 = fm8pool.tile([128, 2, NPIX], FP8, tag="fm8")
                    fm8_tiles.append(f8)
                for c in range(NCHUNK):
                    us = uspool.tile([128, NPIX], FP16, tag="us")
                    q = nc.sync if c % 2 else nc.scalar
                    q.dma_start(out=us[:], in_=fm16[c * 128 : (c + 1) * 128, :])
                    us_tiles.append(us)
                    nc.scalar.activation(
                        out=fm8_tiles[c // 2][:, c % 2, :], in_=us[:], func=AF.Copy
                    )
                    ut = u_pool.tile([128, 2], F32, tag="u")
                    scr_a = scr_pool.tile([128, NPIX], F32, tag="scr_a")
                    ueng.tensor_mul(scr_a[:], us[:], am_bc[:])
                    nc.vector.tensor_reduce(
                        out=ut[:, 0:1], in_=scr_a[:], axis=AX.X, op=OP.add
                    )
                    nc.vector.tensor_reduce(
                        out=ut[:, 1:2], in_=us[:], axis=AX.X, op=OP.add
                    )
                    u16 = u_pool.tile([128, 2], FP16, tag="u16")
                    nc.scalar.activation(out=u16[:], in_=ut[:], func=AF.Copy)
                    u16_tiles.append(u16)

                # ---- phase B: V GEMM (fp8 DR) + s matvec (u^T @ WqT) --------
                with ExitStack() as pb:
                    wv_pool = pb.enter_context(tc.tile_pool(name="wv", bufs=4))
                    wq_pool = pb.enter_context(tc.tile_pool(name="wq", bufs=3))
                    psv = pb.enter_context(tc.tile_pool(name="psv", bufs=4, space="PSUM"))
                    pss = pb.enter_context(tc.tile_pool(name="pss", bufs=4, space="PSUM"))

                    ps_chunks = [
                        pss.tile([2, 512], F32, tag="ps", name=f"ps{_pass}_{k}")
                        for k in range(4)
                    ]
                    for k in range(NCHUNK):
                        # s matvec stream: tiny stationary (u), wide moving (WqT)
                        c = k
                        wqb = wq_pool.tile([128, C], FP16, tag="wq")
                        q = nc.sync if c % 2 == 0 else nc.gpsimd
                        q.dma_start(out=wqb[:], in_=wqT16[c])
                        for ch in range(4):
                            nc.tensor.matmul(
                                ps_chunks[ch][:],
                                u16_tiles[c][:],
                                wqb[:, ch * 512 : (ch + 1) * 512],
                                start=(c == 0),
                                stop=(c == NCHUNK - 1),
                            )

                        # V GEMM stream
                        o = k
                        wvb = wv_pool.tile([128, NCP, 2, 128], FP8, tag="wv")
                        nc.scalar.dma_start(out=wvb[:], in_=wv8[o])
                        pv0 = psv.tile([128, 512], F32, tag="pv")
                        pv1 = psv.tile([128, 512], F32, tag="pv")
                        for cp in range(NCP):
                            nc.tensor.matmul(
                                pv0[:],
                                wvb[:, cp, :, :],
                                fm8_tiles[cp][:, :, 0:512],
                                start=(cp == 0),
                                stop=(cp == NCP - 1),
                                perf_mode=PM.DoubleRow,
                            )
                            nc.tensor.matmul(
                                pv1[:],
                                wvb[:, cp, :, :],
                                fm8_tiles[cp][:, :, 512:1024],
                                start=(cp == 0),
                                stop=(cp == NCP - 1),
                                perf_mode=PM.DoubleRow,
                            )
                        if o % 2 == 0:
                            v8 = vpool.tile([128, 2, VF8], FP8, tag="v8")
                            # ones column (Z) at 1024 in both planes + pad
                            nc.vector.memset(
                                v8[:].bitcast(F32)[:, :, 256 : VF8 // 4], _ONES_PAT
                            )
                            v8_tiles.append(v8)
                        v8 = v8_tiles[o // 2]
                        nc.scalar.activation(
                            out=v8[:, o % 2, 0:512], in_=pv0[:], func=AF.Identity,
                            scale=1.0 / WV_SCALE, bias=bv_t[:, o : o + 1],
                        )
                        nc.scalar.activation(
                            out=v8[:, o % 2, 512:1024], in_=pv1[:], func=AF.Identity,
                            scale=1.0 / WV_SCALE, bias=bv_t[:, o : o + 1],
                        )

                    # evacuate s rows + fold bq in: s += (a_sum, n) * bq
                    for ch in range(4):
                        nc.scalar.activation(
                            out=srow[:, ch * 512 : (ch + 1) * 512],
                            in_=ps_chunks[ch][:], func=AF.Copy,
                        )
                    nc.vector.scalar_tensor_tensor(
                        out=srow[:],
                        in0=bqr_t[:],
                        scalar=scalar2[:, 0:1],
                        in1=srow[:],
                        op0=OP.mult,
                        op1=OP.add,
                    )

            # ---- phase D: exp(scores^T) full rows + probs @ v ---------------
            with ExitStack() as pd:
                rows = pd.enter_context(tc.tile_pool(name="rows", bufs=1))
                e_pool = pd.enter_context(tc.tile_pool(name="e", bufs=NCP))
                es_pool = pd.enter_context(tc.tile_pool(name="es", bufs=2))
                o_pool = pd.enter_context(tc.tile_pool(name="osb", bufs=3))
                t_pool = pd.enter_context(tc.tile_pool(name="t16", bufs=3))
                z_pool = pd.enter_context(tc.tile_pool(name="z", bufs=4))
                pso = pd.enter_context(tc.tile_pool(name="pso", bufs=6, space="PSUM"))

                s1r = rows.tile([128, C], F32, tag="s1r")
                s2r = rows.tile([128, C], F32, tag="s2r")
                m_r = rows.tile([128, C], F32, tag="m_r")
                # partition_broadcast needs base partition 0: bounce s2 row down
                s2row = rows.tile([1, C], F32, tag="s2row")
                nc.sync.dma_start(out=s2row[:], in_=srow[1:2, :])
                nc.gpsimd.partition_broadcast(s1r[:], srow[0:1, :], channels=128)
                nc.gpsimd.partition_broadcast(s2r[:], s2row[:], channels=128)
                # row max via hull support points in partitions + all-reduce
                hxf = es_pool.tile([128, C], F32, tag="escr")
                nc.vector.tensor_scalar_mul(hxf[:], s1r[:], hullc_t[:, 0:1])
                nc.vector.scalar_tensor_tensor(
                    out=hxf[:],
                    in0=s2r[:],
                    scalar=hullc_t[:, 1:2],
                    in1=hxf[:],
                    op0=OP.mult,
                    op1=OP.add,
                )
                nc.gpsimd.partition_all_reduce(
                    m_r[:], hxf[:], channels=128, reduce_op=bass_isa.ReduceOp.max
                )

                eb = []
                for jp in range(NCP):
                    et = e_pool.tile([128, 2, C], FP8, tag="e")
                    for i in range(2):
                        j = 2 * jp + i
                        es = es_pool.tile([128, C], F32, tag="escr")
                        nc.vector.scalar_tensor_tensor(
                            out=es[:],
                            in0=s2r[:],
                            scalar=bk_t[:, j : j + 1],
                            in1=m_r[:],
                            op0=OP.mult,
                            op1=OP.subtract,
                        )
                        nc.vector.scalar_tensor_tensor(
                            out=es[:],
                            in0=s1r[:],
                            scalar=wk_t[:, j : j + 1],
                            in1=es[:],
                            op0=OP.mult,
                            op1=OP.add,
                        )
                        nc.scalar.activation(out=et[:, i, :], in_=es[:], func=AF.Exp)
                    eb.append(et)

                for ig in range(NCHUNK):
                    po = [
                        pso.tile(
                            [128, b - a], F32, tag="po", name=f"po{_pass}_{ig}_{a}"
                        )
                        for (a, b) in NSPLIT8
                    ]
                    for jp in range(NCP):
                        for nidx, (a, b) in enumerate(NSPLIT8):
                            nc.tensor.matmul(
                                po[nidx][:],
                                eb[jp][:, :, ig * 128 : (ig + 1) * 128],
                                v8_tiles[jp][:, :, a:b],
                                start=(jp == 0),
                                stop=(jp == NCP - 1),
                                perf_mode=PM.DoubleRow,
                            )
                    rz = z_pool.tile([128, 1], F32, tag="rz")
                    nc.vector.reciprocal(rz[:], po[2][:, 200:201])
                    rzg = z_pool.tile([128, 1], F32, tag="rzg")
                    nc.vector.tensor_mul(rzg[:], rz[:], gam_bc[:])
                    t16 = t_pool.tile([128, NPIX], FP16, tag="t16")
                    spans = [(0, 412, 0), (412, 824, 1), (824, 1024, 2)]
                    for a, b, nidx in spans:
                        nc.scalar.activation(
                            out=t16[:, a:b], in_=po[nidx][:, 0 : b - a],
                            func=AF.Copy, scale=rzg[:, 0:1],
                        )
                    ot = o_pool.tile([128, NPIX], FP16, tag="ot")
                    nc.vector.tensor_add(ot[:], t16[:], us_tiles[ig][:])
                    nc.sync.dma_start(
                        out=out16[ig * 128 : (ig + 1) * 128, :], in_=ot[:]
                    )

    nc.compile()
    return nc


def host_inputs(feature_map, attention_map, Wq, bq, Wk, bk, Wv, bv, gamma):
    """Shard + lay out inputs for the 8 cores; returns in_maps list."""
    import ml_dtypes

    f32 = np.float32
    B = feature_map.shape[0]
    fm16 = np.ascontiguousarray(feature_map.reshape(B, C, NPIX).astype(np.float16))
    am16 = np.ascontiguousarray(attention_map.reshape(B, 1, NPIX).astype(np.float16))
    # wqT16[c, p, x] = Wq[x, c*128+p]
    wqT16_blk = np.ascontiguousarray(
        Wq.astype(np.float16, copy=False).T.reshape(NCHUNK, 128, C)
    )
    bqrow = np.ascontiguousarray(
        np.broadcast_to(bq.reshape(1, C).astype(f32, copy=False), (2, C))
    )
    # wv8[o, p, cp, i, f] = fp8(64 * Wv[o*128+f, cp*256+i*128+p])
    wv8_blk = np.ascontiguousarray(
        (Wv.astype(f32, copy=False) * WV_SCALE)
        .reshape(NCHUNK, 128, NCP, 2, 128)
        .transpose(0, 4, 2, 3, 1)
        .astype(ml_dtypes.float8_e4m3)
    )
    wk1 = Wk.reshape(C).astype(f32, copy=False)
    bk1 = bk.reshape(C).astype(f32, copy=False)
    smalls = np.ascontiguousarray(
        np.concatenate(
            [
                v.reshape(C).astype(f32, copy=False).reshape(NCHUNK, 128).T
                for v in (Wk, bk, bq, bv)
            ],
            axis=1,
        )
    )

    th = np.arange(NH, dtype=np.float64) * (2.0 * np.pi / NH)
    proj = np.cos(th)[:, None] * wk1[None, :] + np.sin(th)[:, None] * bk1[None, :]
    sel = np.argmax(proj, axis=1)
    sel = np.concatenate([sel, np.full(128 - NH, sel[0])])  # pad partitions
    hullc = np.ascontiguousarray(np.stack([wk1[sel], bk1[sel]], axis=1).astype(f32))

    gam2 = np.ascontiguousarray(gamma.reshape(1, 1).astype(f32, copy=False))

    shared = dict(
        wv8=wv8_blk,
        wqT16=wqT16_blk,
        bqrow=bqrow,
        smalls=smalls,
        hullc=hullc,
        gamma=gam2,
    )
    return [dict(fm16=fm16[b], am16=am16[b], **shared) for b in range(B)]


_NC_CACHE = {}


def get_nc():
    if "nc" not in _NC_CACHE:
        _NC_CACHE["nc"] = build_nc()
    return _NC_CACHE["nc"]


def kernel(feature_map, attention_map, Wq, bq, Wk, bk, Wv, bv, gamma, **run_kwargs):
    from concourse.bass_utils import run_bass_kernel_spmd

    feature_map, attention_map, Wq, bq, Wk, bk, Wv, bv, gamma = (
        np.asarray(x)
        for x in (feature_map, attention_map, Wq, bq, Wk, bk, Wv, bv, gamma)
    )
    B, _, H, W = feature_map.shape
    in_maps = host_inputs(
        feature_map, attention_map, Wq, bq, Wk, bk, Wv, bv, gamma
    )
    nc = get_nc()
    res = run_bass_kernel_spmd(nc, in_maps, core_ids=list(range(NCORES)), **run_kwargs)
    out = np.stack(
        [res.results[b]["out16"].astype(np.float32).reshape(C, H, W) for b in range(B)]
    )
    if run_kwargs:
        kernel.last_results = res
    return out
